# revision 1
# baseline (speedup 1.0000x reference)
"""Trainium2 Bass kernel for the patch-pooling + FF model.

Reference math (per (b, d) slice, x_s = x[b, d] of shape (N=254, L=16)):
    c      = x_s @ W_enc + b_enc + pos_embed          # (N, H)
    csum   = sum_n c                                  # (H,)
    score  = c @ csum (/ sqrt(H), cancels)            # (N,)
    weight = score / sum_n |score|                    # (N,)
    pooled = x_s.T @ weight                           # (L,)
    h      = leaky_relu(pooled @ W1 + b1, 0.2)        # (H,)
    out    = h @ W2 + b2                              # (P,)
returns (out.transpose(0, 2, 1), x)

Key algebra: score_n = c_n . csum = x_n . (W_enc @ csum)
                     + pos_embed_n . csum + b_enc . csum
so c (B,D,N,H) is never materialized. Per core the heavy work is three
elementwise/reduce passes over x plus a handful of small PE matmuls.

Sharding: data-parallel over batch, 4 batches (256 (b,d) slices) per core.
"""

import numpy as np

import concourse.bacc as bacc
import concourse.bass as bass
import concourse.tile as tile
from concourse import mybir
from concourse.bass_utils import run_bass_kernel_spmd

F32 = mybir.dt.float32
AX = mybir.AxisListType
OP = mybir.AluOpType

B, D, N, L, H, P = 32, 64, 254, 16, 128, 96
N_CORES = 8
B_PER = B // N_CORES          # 4 batches per core
S = B_PER * D                 # 256 slices per core
TILE_S = 128                  # slices per SBUF tile (partition dim)
NT = S // TILE_S              # 2 tiles per core
NL = N * L                    # 4064 f32 per slice

# const blob column layout (width per section)
_C_IDENT = 0                  # [128, 128] identity
_C_PET = 128                  # [128, 254] pos_embed.T
_C_WV = 382                   # [128, 17]  [W_enc.T | b_enc]
_C_CBIAS = 399                # [128, 1]   N*b_enc + pos_embed.sum(0)
_C_W2 = 400                   # [128, 96]  W2
_C_B1 = 496                   # [128, 1]   b1
_C_B2 = 497                   # [96, 1]    b2
_C_WENC = 498                 # [16, 128]  W_enc
_C_W1 = 626                   # [16, 128]  W1
C_W = 754                     # total const width


def build_nc():
    nc = bacc.Bacc(
        "TRN2", target_bir_lowering=False, debug=False, num_devices=N_CORES
    )
    x_dram = nc.dram_tensor("x_sh", [S, NL], F32, kind="ExternalInput").ap()
    c_dram = nc.dram_tensor("consts", [128, C_W], F32, kind="ExternalInput").ap()
    o_dram = nc.dram_tensor(
        "out_sh", [B_PER, P, D], F32, kind="ExternalOutput"
    ).ap()

    with tile.TileContext(nc) as tc:
        with (
            tc.tile_pool(name="cpool", bufs=1) as cpool,
            tc.tile_pool(name="xpool", bufs=2) as xpool,
            tc.tile_pool(name="work", bufs=2) as work,
            tc.tile_pool(name="sm", bufs=2) as sm,
            tc.tile_pool(name="psum", bufs=1, space="PSUM") as psum,
        ):
            cb = cpool.tile([128, C_W], F32)
            nc.sync.dma_start(cb[:], c_dram[:, :])
            ident = cb[:, _C_IDENT : _C_IDENT + 128]
            peT = cb[:, _C_PET : _C_PET + N]
            wv = cb[:, _C_WV : _C_WV + L + 1]
            cbias = cb[:, _C_CBIAS : _C_CBIAS + 1]
            w2 = cb[:, _C_W2 : _C_W2 + P]
            b1c = cb[:, _C_B1 : _C_B1 + 1]
            b2c = cb[0:P, _C_B2 : _C_B2 + 1]
            wenc = cb[0:L, _C_WENC : _C_WENC + 128]
            w1 = cb[0:L, _C_W1 : _C_W1 + 128]

            for t in range(NT):
                x_t = xpool.tile([TILE_S, NL], F32)
                nc.sync.dma_start(x_t[:], x_dram[bass.ts(t, TILE_S), :])
                x_nl = x_t[:].rearrange("p (n l) -> p n l", n=N, l=L)
                x_ln = x_t[:].rearrange("p (n l) -> p l n", n=N, l=L)

                # xs[s, l] = sum_n x[s, n, l]
                xs = sm.tile([TILE_S, L], F32)
                nc.vector.tensor_reduce(xs[:], x_ln, axis=AX.X, op=OP.add)

                # xsT[l, s] via PE transpose
                xsT_ps = psum.tile([L, TILE_S], F32)
                nc.tensor.transpose(xsT_ps[:], xs[:], ident)
                xsT = sm.tile([L, TILE_S], F32)
                nc.scalar.copy(xsT[:], xsT_ps[:])

                # csumT[h, s] = W_enc.T @ xsT + (N*b_enc + sum_n pe)
                csum_ps = psum.tile([H, TILE_S], F32)
                nc.tensor.matmul(csum_ps[:], wenc, xsT[:])
                csumT = sm.tile([H, TILE_S], F32)
                nc.vector.tensor_scalar_add(csumT[:], csum_ps[:], cbias)

                # vb[s, 0:16] = v = W_enc @ csum ; vb[s, 16] = b_enc . csum
                vb_ps = psum.tile([TILE_S, L + 1], F32)
                nc.tensor.matmul(vb_ps[:], csumT[:], wv)
                vb = sm.tile([TILE_S, L + 1], F32)
                nc.scalar.copy(vb[:], vb_ps[:])

                # spe[s, n] = pos_embed_n . csum_s
                spe_ps = psum.tile([TILE_S, N], F32)
                nc.tensor.matmul(spe_ps[:], csumT[:], peT)

                # sdot[s, n] = x_n . v_s
                prod = work.tile([TILE_S, NL], F32)
                prod_nl = prod[:].rearrange("p (n l) -> p n l", n=N, l=L)
                v_bc = vb[:, 0:L].unsqueeze(1).broadcast_to((TILE_S, N, L))
                nc.vector.tensor_mul(prod_nl, x_nl, v_bc)
                sdot = sm.tile([TILE_S, N], F32)
                nc.vector.tensor_reduce(sdot[:], prod_nl, axis=AX.X, op=OP.add)

                # score = sdot + spe + bdot  (1/sqrt(H) cancels in weight)
                sc0 = sm.tile([TILE_S, N], F32)
                nc.vector.tensor_add(sc0[:], sdot[:], spe_ps[:])
                score = sm.tile([TILE_S, N], F32)
                nc.vector.tensor_scalar_add(score[:], sc0[:], vb[:, L : L + 1])

                # weight = score / sum_n |score|
                sabs = sm.tile([TILE_S, 1], F32)
                nc.vector.tensor_reduce(
                    sabs[:], score[:], axis=AX.X, op=OP.add,
                    apply_absolute_value=True,
                )
                rec = sm.tile([TILE_S, 1], F32)
                nc.vector.reciprocal(rec[:], sabs[:])
                weight = sm.tile([TILE_S, N], F32)
                nc.vector.tensor_scalar_mul(weight[:], score[:], rec[:])

                # pooled[s, l] = sum_n weight[s, n] * x[s, n, l]
                prod2 = work.tile([TILE_S, NL], F32)
                prod2_ln = prod2[:].rearrange("p (n l) -> p l n", n=N, l=L)
                prod2_nl = prod2[:].rearrange("p (n l) -> p n l", n=N, l=L)
                w_bc = weight[:].unsqueeze(2).broadcast_to((TILE_S, N, L))
                nc.vector.tensor_mul(prod2_nl, x_nl, w_bc)
                pooled = sm.tile([TILE_S, L], F32)
                nc.vector.tensor_reduce(pooled[:], prod2_ln, axis=AX.X, op=OP.add)

                # FF: h = leaky_relu(pooled @ W1 + b1, 0.2); out = h @ W2 + b2
                pT_ps = psum.tile([L, TILE_S], F32)
                nc.tensor.transpose(pT_ps[:], pooled[:], ident)
                pT = sm.tile([L, TILE_S], F32)
                nc.scalar.copy(pT[:], pT_ps[:])

                h_ps = psum.tile([H, TILE_S], F32)
                nc.tensor.matmul(h_ps[:], w1, pT[:])
                hb = sm.tile([H, TILE_S], F32)
                nc.vector.tensor_scalar_add(hb[:], h_ps[:], b1c)
                hs = sm.tile([H, TILE_S], F32)
                nc.vector.tensor_scalar_mul(hs[:], hb[:], 0.2)
                h_sb = sm.tile([H, TILE_S], F32)
                nc.vector.tensor_tensor(h_sb[:], hb[:], hs[:], op=OP.max)

                o_ps = psum.tile([P, TILE_S], F32)
                nc.tensor.matmul(o_ps[:], w2, h_sb[:])
                o_sb = sm.tile([P, TILE_S], F32)
                nc.vector.tensor_scalar_add(o_sb[:], o_ps[:], b2c)

                for k in range(TILE_S // D):
                    b_loc = t * (TILE_S // D) + k
                    nc.sync.dma_start(
                        o_dram[b_loc], o_sb[:, bass.ts(k, D)]
                    )

    nc.compile()
    return nc


_NC_CACHE = None


def _get_nc():
    global _NC_CACHE
    if _NC_CACHE is None:
        _NC_CACHE = build_nc()
    return _NC_CACHE


def _make_consts(W_enc, b_enc, W1, b1, W2, b2, pos_embed):
    cb = np.zeros((128, C_W), dtype=np.float32)
    cb[:, _C_IDENT : _C_IDENT + 128] = np.eye(128, dtype=np.float32)
    cb[:, _C_PET : _C_PET + N] = pos_embed.T
    cb[:, _C_WV : _C_WV + L] = W_enc.T
    cb[:, _C_WV + L] = b_enc
    cb[:, _C_CBIAS] = N * b_enc + pos_embed.sum(axis=0)
    cb[:, _C_W2 : _C_W2 + P] = W2
    cb[:, _C_B1] = b1
    cb[0:P, _C_B2] = b2
    cb[0:L, _C_WENC : _C_WENC + 128] = W_enc
    cb[0:L, _C_W1 : _C_W1 + 128] = W1
    return cb


def kernel(x, W_enc, b_enc, W1, b1, W2, b2, pos_embed):
    x = np.ascontiguousarray(np.asarray(x, dtype=np.float32))
    consts = _make_consts(
        np.asarray(W_enc, np.float32), np.asarray(b_enc, np.float32),
        np.asarray(W1, np.float32), np.asarray(b1, np.float32),
        np.asarray(W2, np.float32), np.asarray(b2, np.float32),
        np.asarray(pos_embed, np.float32),
    )
    nc = _get_nc()
    in_maps = []
    for c in range(N_CORES):
        x_sh = x[c * B_PER : (c + 1) * B_PER].reshape(S, NL)
        in_maps.append({"x_sh": np.ascontiguousarray(x_sh), "consts": consts})
    res = run_bass_kernel_spmd(nc, in_maps, list(range(N_CORES)))
    out = np.concatenate(
        [res.results[c]["out_sh"] for c in range(N_CORES)], axis=0
    )  # (B, P, D)
    return out, x
